# revision 2
# baseline (speedup 1.0000x reference)
"""Trainium2 Bass kernel for capsule dynamic routing (nn_Capsule).

Reference (per batch item b):
    u = x_b @ W; logits = 0
    for i in 4:
        c = softmax(logits, axis=capsule)
        t_j = sum_s c[s,j] * u[s, j*64:(j+1)*64]; v = squash(t)
        if i < 3: logits[s,j] += u[s, jblk] . v_j

This kernel never materializes u. By linearity:
    y_j = sum_s c[s,j] x_s          (GEMM over S, X natural layout)
    t_j = y_j @ W_jblk              (block-diag of  W^T y^T)
    P_j = W_jblk @ v_j              (P = W^T-transposed GEMM vs blockdiag v)
    upd = X @ P                     (GEMM over H, X^T layout)
~10x fewer FLOPs than materializing u.

Sharding: batch-parallel, 8 batch items per core, W replicated.
fp16 matmul operands (fp32 PSUM accumulation, fp32 logits/softmax).

HW lessons encoded here:
  - DVE copy PSUM(f32) -> SBUF(f16) is a device-killing op; use ScalarE
    activation(Copy) for every PSUM -> f16 cast.
  - Each PE-transpose output needs its own (bank-padded) PSUM tile;
    packing several transpose outputs into one PSUM tile faults the device.
  - matmul start=True lazily zeroes the full 2KB PSUM bank for the out
    AP's partitions: accumulation groups must own a (partition-range x
    bank) region exclusively. Partition-disjoint groups interleave fine
    (sim's group check is partition-agnostic: skip_group_check=True).
  - nc.vector.memset values on f16 tiles are unreliable: load constants
    from host DRAM instead.
"""
import numpy as np
from contextlib import ExitStack

import concourse.bass as bass
import concourse.bacc as bacc
import concourse.tile as tile
from concourse import mybir
from concourse.bass_utils import run_bass_kernel_spmd

f16 = mybir.dt.float16
f32 = mybir.dt.float32
COPY = mybir.ActivationFunctionType.Copy
EXP = mybir.ActivationFunctionType.Exp
SQRT = mybir.ActivationFunctionType.Sqrt

S, B, H = 512, 64, 1024
NCAP, DCAP = 16, 64
ROUTINGS = 4
N_CORES = 8
BL = B // N_CORES          # 8 batch items per core
SC = S // 128              # 4 s-chunks
HC = H // 128              # 8 h-chunks
OC = H // 128              # 8 o-chunks (o = NCAP*DCAP = 1024)


def _act_copy(nc, out, in_):
    nc.scalar.activation(out=out, in_=in_, func=COPY, scale=1.0, alpha=0.0)


def _build_kernel(tc, out_d, x_d, w_d, id_d, o2t_d, ones2_d, eps_d,
                  cpad_d, logits_d, vblk_d):
    nc = tc.nc
    ctx = ExitStack()
    const = ctx.enter_context(tc.tile_pool(name="const", bufs=1))
    work = ctx.enter_context(tc.tile_pool(name="work", bufs=2))
    small = ctx.enter_context(tc.tile_pool(name="small", bufs=2))
    ps_big = ctx.enter_context(tc.tile_pool(name="ps_big", bufs=2, space="PSUM"))
    ps_sm = ctx.enter_context(tc.tile_pool(name="ps_sm", bufs=2, space="PSUM"))
    ps_tp = ctx.enter_context(tc.tile_pool(name="ps_tp", bufs=2, space="PSUM"))

    # ---------- persistent tensors ----------
    x16 = const.tile([128, BL, SC, 1024], f16)    # X natural (s_loc, b, sc, h)
    xt16 = const.tile([128, BL, HC, 512], f16)    # X^T (h_loc, b, hc, s)
    w16 = const.tile([128, HC, 1024], f16)        # W natural (h_loc, hc, o)
    wt16 = const.tile([128, OC, 1024], f16)       # W^T (o_loc, oc, h)
    id16 = const.tile([128, 128], f16)            # eye(128)
    id32 = const.tile([128, 128], f32)
    ones2 = const.tile([128, 2], f16)             # [[1;0],[0;1]] halves
    o2t = const.tile([2, 128], f32)               # broadcast helper
    eps = const.tile([2, 1], f32)
    cpad = const.tile([128, BL, SC, 32], f16)     # c, cols 16-31 zero pad
    logits = const.tile([128, BL, SC, 16], f32)
    vblk = const.tile([128, OC, 256], f16)        # block-diag v, (b,32) pad

    # ---------- loads (SWDGE casts f32 -> f16 where needed) ----------
    wr = w_d.rearrange("(hc p) o -> p hc o", p=128)
    nc.gpsimd.dma_start(out=w16[:], in_=wr)
    for hc in range(HC):
        # (128 h, 1024 o) -> (128 o_loc, 8 oc, 128 h-slice)
        nc.sync.dma_start_transpose(
            wt16[:, :, hc * 128:(hc + 1) * 128], w16[:, hc, :])
    nc.gpsimd.dma_start(out=id16[:], in_=id_d[:])
    nc.gpsimd.dma_start(out=id32[:], in_=id_d[:])
    nc.gpsimd.dma_start(out=o2t[:], in_=o2t_d[:])
    nc.gpsimd.dma_start(out=ones2[:], in_=ones2_d[:])
    nc.gpsimd.dma_start(out=eps[:], in_=eps_d[:])
    nc.gpsimd.dma_start(out=cpad[:], in_=cpad_d[:])
    nc.gpsimd.dma_start(out=logits[:], in_=logits_d[:])
    nc.gpsimd.dma_start(out=vblk[:], in_=vblk_d[:])
    xr = x_d.rearrange("(sc p) b h -> p b sc h", p=128)
    for b in range(BL):
        nc.gpsimd.dma_start(out=x16[:, b, :, :], in_=xr[:, b, :, :])
    for b in range(BL):
        for sc in range(SC):
            # (128 s, 1024 h) -> (128 h_loc, 8 hc, 128 s-slice)
            nc.sync.dma_start_transpose(
                xt16[:, b, :, sc * 128:sc * 128 + 128], x16[:, b, sc, :])

    v32 = None
    for it in range(ROUTINGS):
        last = it == ROUTINGS - 1

        # ---------- y = C^T X: out (4b x 32pad, 1024) f32 per group -------
        y_ps = [ps_big.tile([128, 1024], f32, tag="big", name=f"y_ps{g}")
                for g in range(2)]
        for g in range(2):
            for sc in range(SC):
                for half in range(2):
                    for b_ in range(4):
                        b = 4 * g + b_
                        nc.tensor.matmul(
                            y_ps[g][32 * b_:32 * b_ + 32,
                                    512 * half:512 * half + 512],
                            cpad[:, b, sc, :],
                            x16[:, b, sc, 512 * half:512 * half + 512],
                            start=(sc == 0), stop=(sc == SC - 1),
                            skip_group_check=True,
                            tile_position=(0, 32 * b_))
        y_sb = work.tile([128, 2, 1024], f16, tag="y_sb")
        for g in range(2):
            _act_copy(nc, y_sb[:, g, :], y_ps[g][:])

        # ---------- y^T via PE transposes (own PSUM tile each) ----------
        yt_sb = work.tile([128, HC, 256], f16, tag="yt_sb")
        for hc in range(HC):
            for g in range(2):
                tp = ps_tp.tile([128, 128], f16, tag="tp",
                                name=f"yt_tp{hc}_{g}")
                nc.tensor.matmul(
                    tp[:], y_sb[:, g, 128 * hc:128 * hc + 128], id16[:],
                    is_transpose=True, skip_group_check=True)
                nc.vector.tensor_copy(
                    yt_sb[:, hc, 128 * g:128 * g + 128], tp[:])

        # ---------- T^T = W^T y^T, extract block-diag into t ----------
        t_sb = small.tile([128, 64], f32, tag="t_sb")  # (jp*64+d, b*8+oc)
        for oc in range(OC):
            t_ps = ps_sm.tile([128, 256], f32, tag="sm", name=f"t_ps{oc}")
            for hc in range(HC):
                nc.tensor.matmul(
                    t_ps[:],
                    w16[:, hc, 128 * oc:128 * oc + 128],
                    yt_sb[:, hc, :],
                    start=(hc == 0), stop=(hc == HC - 1))
            src = t_ps.rearrange("p (b j) -> p b j", j=32)
            dst = t_sb.rearrange("p (b o) -> p b o", o=8)
            nc.vector.tensor_copy(dst[0:64, :, oc], src[0:64, :, 2 * oc])
            nc.vector.tensor_copy(dst[64:128, :, oc],
                                  src[64:128, :, 2 * oc + 1])

        # ---------- squash: v = t / sqrt(sum_d t^2 + eps) ----------
        t2 = small.tile([128, 64], f16, tag="t2")
        nc.vector.tensor_mul(t2[:], t_sb[:], t_sb[:])
        sq_ps = ps_sm.tile([2, 64], f32, tag="sm", name="sq_ps")
        nc.tensor.matmul(sq_ps[:], ones2[:], t2[:])
        sq_sb = small.tile([2, 64], f32, tag="sq_sb")
        nc.scalar.activation(out=sq_sb[:], in_=sq_ps[:], func=SQRT,
                             bias=eps[:], scale=1.0, alpha=0.0)
        rs = small.tile([2, 64], f32, tag="rs")
        nc.vector.reciprocal(rs[:], sq_sb[:])
        # broadcast rs (2,64) -> (128,64): bc[p,n] = rs[p//64, n]
        bc_ps = ps_sm.tile([128, 64], f32, tag="sm", name="bc_ps")
        nc.tensor.matmul(bc_ps[:], o2t[:], rs[:])
        bc_sb = small.tile([128, 64], f32, tag="bc_sb")
        nc.vector.tensor_copy(bc_sb[:], bc_ps[:])
        if last:
            v32 = small.tile([128, 64], f32, tag="v32")
            nc.vector.tensor_mul(v32[:], t_sb[:], bc_sb[:])
            break
        v16 = small.tile([128, 64], f16, tag="v16")
        nc.vector.tensor_mul(v16[:], t_sb[:], bc_sb[:])

        # ---------- scatter v into block-diag vblk ----------
        vv = v16.rearrange("p (b o) -> p b o", o=8)
        for oc in range(OC):
            dstv = vblk[:, oc, :].rearrange("p (b j) -> p b j", j=32)
            nc.vector.tensor_copy(dstv[0:64, :, 2 * oc], vv[0:64, :, oc])
            nc.vector.tensor_copy(dstv[64:128, :, 2 * oc + 1],
                                  vv[64:128, :, oc])

        # ---------- P = W Vblk ----------
        p_sb = work.tile([128, HC, 256], f16, tag="p_sb")
        for hc in range(HC):
            p_ps = ps_sm.tile([128, 256], f32, tag="sm", name=f"p_ps{hc}")
            for oc in range(OC):
                nc.tensor.matmul(
                    p_ps[:],
                    wt16[:, oc, 128 * hc:128 * hc + 128],
                    vblk[:, oc, :],
                    start=(oc == 0), stop=(oc == OC - 1))
            _act_copy(nc, p_sb[:, hc, :], p_ps[:])

        # ---------- update = X P via X^T ----------
        u_ps = [ps_sm.tile([128, 512], f32, tag="sm", name=f"u_ps{g}")
                for g in range(2)]
        for g in range(2):
            for hc in range(HC):
                for b_ in range(4):
                    b = 4 * g + b_
                    nc.tensor.matmul(
                        u_ps[g][32 * b_:32 * b_ + 32, :],
                        p_sb[:, hc, 32 * b:32 * b + 32],
                        xt16[:, b, hc, :],
                        start=(hc == 0), stop=(hc == HC - 1),
                        skip_group_check=True,
                        tile_position=(0, 32 * b_))
        u_sb = work.tile([128, 2, 512], f32, tag="u_sb")
        for g in range(2):
            nc.vector.tensor_copy(u_sb[:, g, :], u_ps[g][:])

        # ---------- transpose update, accumulate logits ----------
        for sc in range(SC):
            for g in range(2):
                utp = ps_tp.tile([128, 128], f32, tag="tp",
                                 name=f"ut_tp{sc}_{g}")
                nc.tensor.matmul(
                    utp[:], u_sb[:, g, 128 * sc:128 * sc + 128], id32[:],
                    is_transpose=True, skip_group_check=True)
                src = utp.rearrange("p (b j) -> p b j", j=32)
                nc.vector.tensor_add(
                    logits[:, 4 * g:4 * g + 4, sc, :],
                    logits[:, 4 * g:4 * g + 4, sc, :], src[:, :, 0:16])

        # ---------- softmax over capsules -> cpad ----------
        for sc in range(SC):
            ex = small.tile([128, BL, 16], f32, tag="ex")
            nc.scalar.activation(out=ex[:], in_=logits[:, :, sc, :],
                                 func=EXP, scale=1.0, alpha=0.0)
            sm = small.tile([128, BL, 1], f32, tag="sm")
            nc.vector.reduce_sum(sm[:], ex[:], axis=mybir.AxisListType.X)
            rc = small.tile([128, BL, 1], f32, tag="rc")
            nc.vector.reciprocal(rc[:], sm[:])
            nc.vector.tensor_mul(cpad[:, :, sc, 0:16], ex[:],
                                 rc.broadcast_to([128, BL, 16]))

    # ---------- out[b, 2*oc+jp, d] = v32[jp*64+d, b*8+oc] ----------
    out_ap = bass.AP(tensor=out_d.tensor, offset=0,
                     ap=[[1, 128], [1024, BL], [128, 8]])
    nc.sync.dma_start(out=out_ap, in_=v32.rearrange("p (b o) -> p b o", o=8))
    ctx.close()


_CACHE = {}


def _host_consts():
    ident = np.ascontiguousarray(np.eye(128, dtype=np.float32))
    o2t = np.zeros((2, 128), np.float32)
    o2t[0, 0:64] = 1.0
    o2t[1, 64:128] = 1.0
    ones2 = np.zeros((128, 2), np.float32)
    ones2[0:64, 0] = 1.0
    ones2[64:128, 1] = 1.0
    eps = np.full((2, 1), 1e-7, np.float32)
    cpad = np.zeros((128, BL, SC, 32), np.float32)
    cpad[:, :, :, 0:16] = 1.0 / NCAP          # iteration-0 softmax is exact
    logi = np.zeros((128, BL, SC, 16), np.float32)
    vblk = np.zeros((128, OC, 256), np.float32)
    return {"ident": ident, "o2t": o2t, "ones2": ones2, "eps": eps,
            "cpadi": cpad, "logi": logi, "vblki": vblk}


def _get_nc():
    if "nc" not in _CACHE:
        nc = bacc.Bacc("TRN2", target_bir_lowering=False, debug=False)
        x_d = nc.dram_tensor("x", [S, BL, H], f32, kind="ExternalInput")
        w_d = nc.dram_tensor("w", [H, NCAP * DCAP], f32, kind="ExternalInput")
        id_d = nc.dram_tensor("ident", [128, 128], f32, kind="ExternalInput")
        o2t_d = nc.dram_tensor("o2t", [2, 128], f32, kind="ExternalInput")
        ones2_d = nc.dram_tensor("ones2", [128, 2], f32, kind="ExternalInput")
        eps_d = nc.dram_tensor("eps", [2, 1], f32, kind="ExternalInput")
        cpad_d = nc.dram_tensor("cpadi", [128, BL, SC, 32], f32,
                                kind="ExternalInput")
        logits_d = nc.dram_tensor("logi", [128, BL, SC, 16], f32,
                                  kind="ExternalInput")
        vblk_d = nc.dram_tensor("vblki", [128, OC, 256], f32,
                                kind="ExternalInput")
        out_d = nc.dram_tensor("out", [BL, NCAP, DCAP], f32,
                               kind="ExternalOutput")
        with tile.TileContext(nc) as tc:
            _build_kernel(tc, out_d.ap(), x_d.ap(), w_d.ap(), id_d.ap(),
                          o2t_d.ap(), ones2_d.ap(), eps_d.ap(), cpad_d.ap(),
                          logits_d.ap(), vblk_d.ap())
        nc.compile()
        _CACHE["nc"] = nc
    return _CACHE["nc"]


def kernel(inputs: np.ndarray, W: np.ndarray, _trace: bool = False):
    """inputs: (512, 64, 1024) f32; W: (1, 1024, 1024) f32.
    Returns (64, 16, 64) f32."""
    nc = _get_nc()
    consts = _host_consts()
    wf = np.ascontiguousarray(W[0].astype(np.float32))
    in_maps = []
    for c in range(N_CORES):
        m = {"x": np.ascontiguousarray(
                 inputs[:, c * BL:(c + 1) * BL, :].astype(np.float32)),
             "w": wf}
        m.update(consts)
        in_maps.append(m)
    kw = {}
    if _trace:
        kw = dict(trace=True, trace_cores=list(range(N_CORES)),
                  stitch_traces=False)
    res = run_bass_kernel_spmd(nc, in_maps, core_ids=list(range(N_CORES)),
                               **kw)
    out = np.concatenate([res.results[c]["out"] for c in range(N_CORES)],
                         axis=0)
    if _trace:
        return out.astype(np.float32), res
    return out.astype(np.float32)



# revision 14
# speedup vs baseline: 1.4344x; 1.4344x over previous
"""Trainium2 Bass kernel for capsule dynamic routing (nn_Capsule).

Reference (per batch item b):
    u = x_b @ W; logits = 0
    for i in 4:
        c = softmax(logits, axis=capsule)
        t_j = sum_s c[s,j] * u[s, j*64:(j+1)*64]; v = squash(t)
        if i < 3: logits[s,j] += u[s, jblk] . v_j

Never materializes u. By linearity:
    y_q   = sum_s c[s,q] x_s        (GEMM over S, X natural layout)
    T     = y @ W, t = blockdiag(T) (dense q=(b,j) partition layout)
    P     = W^T' Vblk               (Vblk = block-diag of v)
    upd   = X P                     (GEMM over H, X^T layout)

Sharding: batch-parallel, 8 batch items per core, W replicated.
Dense partition index q = b*16 + j (8 batch x 16 capsules = 128).

Perf design (vs. the first working version):
  - All big operands are pre-packed on HOST into the exact f16 SBUF
    layouts (x, x^T, W, W^T) -> contiguous HWDGE DMA loads, no SWDGE
    casts, no element-strided on-chip transpose DMAs.
  - T and P are computed in the dense q layout: 16 matmuls of
    N=512/128 per step instead of 64 padded-256 matmuls.
  - squash runs in natural layout (q on partitions, d on free axis):
    DVE reduce + ACT sqrt, no PE broadcast matmuls.

HW lessons kept from the previous version:
  - ScalarE activation(Copy) for every PSUM f32 -> f16 cast (DVE dies).
  - Each PE-transpose output gets its own PSUM tile.
  - matmul start=True lazily zeroes the PSUM bank for the out AP's
    partitions; partition-disjoint groups interleave with
    skip_group_check=True.
  - f16 constants come from host DRAM, not memset.
"""
import numpy as np
from contextlib import ExitStack

import concourse.bass as bass
import concourse.bacc as bacc
import concourse.tile as tile
from concourse import mybir
from concourse.bass_utils import run_bass_kernel_spmd

f16 = mybir.dt.float16
f32 = mybir.dt.float32
COPY = mybir.ActivationFunctionType.Copy
EXP = mybir.ActivationFunctionType.Exp
SQRT = mybir.ActivationFunctionType.Sqrt

S, B, H = 512, 64, 1024
NCAP, DCAP = 16, 64
ROUTINGS = 4
N_CORES = 8
BL = B // N_CORES          # 8 batch items per core
SC = S // 128              # 4 s-chunks
HC = H // 128              # 8 h-chunks
OC = H // 128              # 8 o-chunks (o = NCAP*DCAP = 1024)


def _act_copy(nc, out, in_):
    nc.scalar.activation(out=out, in_=in_, func=COPY, scale=1.0, alpha=0.0)


def _build_kernel(tc, out_d, x_d, xt_d, w_d, wt_d, id16_d, id32_d, eps_d,
                  cpad_d, logits_d, vblk_d, tt_d):
    nc = tc.nc
    ctx = ExitStack()
    const = ctx.enter_context(tc.tile_pool(name="const", bufs=1))
    work = ctx.enter_context(tc.tile_pool(name="work", bufs=2))
    small = ctx.enter_context(tc.tile_pool(name="small", bufs=2))
    # PSUM budget: 8 banks. big: y_ps/t_ps/u_ps cycle one 4-bank slot;
    # sm: p_ps 2 x 1 bank; tp: all transposes 2 x 1 bank.
    ps_big = ctx.enter_context(tc.tile_pool(name="ps_big", bufs=1,
                                            space="PSUM"))
    ps_sm = ctx.enter_context(tc.tile_pool(name="ps_sm", bufs=2, space="PSUM"))
    ps_tp = ctx.enter_context(tc.tile_pool(name="ps_tp", bufs=2, space="PSUM"))

    # ---------- persistent tensors ----------
    x16 = const.tile([128, BL, SC, 1024], f16)    # X natural (s_loc, b, sc, h)
    xt16 = const.tile([128, BL, HC, 512], f16)    # X^T (h_loc, b, hc, s)
    w16 = const.tile([128, HC, 1024], f16)        # W natural (h_loc, hc, o)
    wt16 = const.tile([128, OC, 1024], f16)       # W^T (o_loc, oc, h)
    id16 = const.tile([128, 128], f16)            # eye(128) f16
    id32 = const.tile([128, 128], f32)            # eye(128) f32
    eps = const.tile([128, 1], f32)
    cpad = const.tile([128, BL, SC, 32], f16)     # c, cols 16-31 zero pad
    logits = const.tile([128, BL, SC, 16], f32)
    vblk = const.tile([128, OC, 128], f16)        # block-diag v, dense q cols
    tt32 = const.tile([128, 128], f32)            # t^T split, zeros elsewhere

    # ---------- loads (all contiguous, host-prepacked f16) ----------
    for sc in range(SC):
        for b in range(BL):
            nc.sync.dma_start(out=x16[:, b, sc, :], in_=x_d[:, b, sc, :])
    nc.sync.dma_start(out=w16[:], in_=w_d[:])
    nc.sync.dma_start(out=wt16[:], in_=wt_d[:])
    for b in range(BL):
        nc.sync.dma_start(out=xt16[:, b, :, :], in_=xt_d[:, b, :, :])
    nc.sync.dma_start(out=id16[:], in_=id16_d[:])
    nc.sync.dma_start(out=id32[:], in_=id32_d[:])
    nc.sync.dma_start(out=eps[:], in_=eps_d[:])
    nc.sync.dma_start(out=cpad[:], in_=cpad_d[:])
    nc.sync.dma_start(out=logits[:], in_=logits_d[:])
    nc.sync.dma_start(out=vblk[:], in_=vblk_d[:])
    nc.sync.dma_start(out=tt32[:], in_=tt_d[:])

    v32 = None
    for it in range(ROUTINGS):
        last = it == ROUTINGS - 1

        # ---------- y = C^T X: (2g x 4b x 32pad part, 1024 h) f32 ----------
        y_ps = ps_big.tile([128, 2, 1024], f32, tag="big", name=f"y_ps{it}")
        for g in range(2):
            for half in range(2):
                for sc in range(SC):
                    for b_ in range(4):
                        b = 4 * g + b_
                        nc.tensor.matmul(
                            y_ps[32 * b_:32 * b_ + 32, g,
                                 512 * half:512 * half + 512],
                            cpad[:, b, sc, :],
                            x16[:, b, sc, 512 * half:512 * half + 512],
                            start=(sc == 0), stop=(sc == SC - 1),
                            skip_group_check=True,
                            tile_position=(0, 32 * b_))
        y_sb = work.tile([128, 2, 1024], f16, tag="y_sb")
        for g in range(2):
            for half in range(2):
                _act_copy(nc, y_sb[:, g, 512 * half:512 * half + 512],
                          y_ps[:, g, 512 * half:512 * half + 512])

        # ---------- y^T via PE transposes, dense-packed q cols ----------
        yt = work.tile([128, HC, 128], f16, tag="yt")
        for hc in range(HC):
            for g in range(2):
                tp = ps_tp.tile([128, 128], f16, tag="tp",
                                name=f"yt_tp{it}_{hc}_{g}")
                nc.tensor.matmul(
                    tp[:], y_sb[:, g, 128 * hc:128 * hc + 128], id16[:],
                    is_transpose=True, skip_group_check=True)
                # dense capsule-major pack: yt col q = j*8 + b
                src = tp.rearrange("p (b j) -> p j b", j=32)[:, 0:16, :]
                dst = yt[:, hc, :].rearrange(
                    "p (j b) -> p j b", b=8)[:, :, 4 * g:4 * g + 4]
                if (hc + 2 * g) % 2 == 0:
                    nc.vector.tensor_copy(dst, src)
                else:
                    _act_copy(nc, dst, src)

        # ---------- T^T = W^T y^T per oc chunk; extract diag blocks ------
        # tt32[par*64 + d, q] = t[q, d] for q in [8j, 8j+8), j = 2oc+par.
        # All other cells stay zero (host-initialized) so squash and the
        # v-scatter can run on full 128-wide tiles.
        for oc in range(OC):
            ttp = ps_sm.tile([128, 128], f32, tag="sm",
                             name=f"ttp{it}_{oc}")
            for hc in range(HC):
                nc.tensor.matmul(
                    ttp[:],
                    w16[:, hc, 128 * oc:128 * oc + 128],
                    yt[:, hc, :],
                    start=(hc == 0), stop=(hc == HC - 1))
            for par in range(2):
                j = 2 * oc + par
                p0 = 64 * par
                nc.vector.tensor_copy(
                    tt32[p0:p0 + 64, 8 * j:8 * j + 8],
                    ttp[p0:p0 + 64, 8 * j:8 * j + 8])

        # ---------- transpose to t[q, d'] (d' = d + 64*(j%2)) ----------
        ttq = ps_tp.tile([128, 128], f32, tag="tp", name=f"ttq{it}")
        nc.tensor.matmul(ttq[:], tt32[:], id32[:],
                         is_transpose=True, skip_group_check=True)
        t_sb = small.tile([128, 128], f32, tag="t_sb")
        nc.vector.tensor_copy(t_sb[:], ttq[:])

        # ---------- squash: v = t / sqrt(sum_d t^2 + eps) ----------
        t2 = small.tile([128, 128], f32, tag="t2")
        nc.vector.tensor_mul(t2[:], t_sb[:], t_sb[:])
        ssum = small.tile([128, 1], f32, tag="ssum")
        nc.vector.reduce_sum(ssum[:], t2[:], axis=mybir.AxisListType.X)
        snorm = small.tile([128, 1], f32, tag="snorm")
        nc.scalar.activation(out=snorm[:], in_=ssum[:], func=SQRT,
                             bias=eps[:], scale=1.0, alpha=0.0)
        rs = small.tile([128, 1], f32, tag="rs")
        nc.vector.reciprocal(rs[:], snorm[:])
        if last:
            v32 = small.tile([128, 128], f32, tag="v32")
            nc.vector.tensor_mul(v32[:], t_sb[:],
                                 rs.broadcast_to([128, 128]))
            break
        v16 = small.tile([128, 128], f16, tag="v16")
        nc.vector.tensor_mul(v16[:], t_sb[:], rs.broadcast_to([128, 128]))

        # ---------- scatter v into block-diag vblk (via transpose) -------
        vtp = ps_tp.tile([128, 128], f16, tag="tp", name=f"v_tp{it}")
        nc.tensor.matmul(vtp[:], v16[:], id16[:],
                         is_transpose=True, skip_group_check=True)
        for oc in range(OC):
            for par in range(2):
                j = 2 * oc + par
                p0, p1 = 64 * par, 64 * par + 64
                nc.vector.tensor_copy(vblk[p0:p1, oc, 8 * j:8 * j + 8],
                                      vtp[p0:p1, 8 * j:8 * j + 8])

        # ---------- P = W Vblk: (128 h_loc, hc, 128 q) f16 ----------
        p_sb = work.tile([128, HC, 128], f16, tag="p_sb")
        for hc in range(HC):
            p_ps = ps_sm.tile([128, 128], f32, tag="sm", name=f"p_ps{it}_{hc}")
            for oc in range(OC):
                nc.tensor.matmul(
                    p_ps[:],
                    wt16[:, oc, 128 * hc:128 * hc + 128],
                    vblk[:, oc, :],
                    start=(oc == 0), stop=(oc == OC - 1))
            # permute cols capsule-major q=j*8+b -> batch-major b*16+j so
            # the update step gets contiguous per-batch weight slices
            _act_copy(nc,
                      p_sb[:, hc, :].rearrange("p (b j) -> p j b", j=16),
                      p_ps.rearrange("p (j b) -> p j b", b=8))

        # ---------- update = X P via X^T (dense 16-col weights) ----------
        u_ps = ps_big.tile([128, 2, 512], f32, tag="big", name=f"u_ps{it}")
        for g in range(2):
            for hc in range(HC):
                for b_ in range(4):
                    b = 4 * g + b_
                    nc.tensor.matmul(
                        u_ps[32 * b_:32 * b_ + 16, g, :],
                        p_sb[:, hc, 16 * b:16 * b + 16],
                        xt16[:, b, hc, :],
                        start=(hc == 0), stop=(hc == HC - 1),
                        skip_group_check=True,
                        tile_position=(0, 32 * b_))
        u_sb = work.tile([128, 2, 512], f32, tag="u_sb")
        for g in range(2):
            nc.vector.tensor_copy(u_sb[:, g, :], u_ps[:, g, :])

        # ---------- transpose update, accumulate logits ----------
        for sc in range(SC):
            for g in range(2):
                utp = ps_tp.tile([128, 128], f32, tag="tp",
                                 name=f"ut_tp{it}_{sc}_{g}")
                nc.tensor.matmul(
                    utp[:], u_sb[:, g, 128 * sc:128 * sc + 128], id32[:],
                    is_transpose=True, skip_group_check=True)
                src = utp.rearrange("p (b j) -> p b j", j=32)
                nc.vector.tensor_add(
                    logits[:, 4 * g:4 * g + 4, sc, :],
                    logits[:, 4 * g:4 * g + 4, sc, :], src[:, :, 0:16])

        # ---------- softmax over capsules -> cpad ----------
        for sc in range(SC):
            ex = small.tile([128, BL, 16], f32, tag="ex")
            nc.scalar.activation(out=ex[:], in_=logits[:, :, sc, :],
                                 func=EXP, scale=1.0, alpha=0.0)
            sm = small.tile([128, BL, 1], f32, tag="sm")
            nc.vector.reduce_sum(sm[:], ex[:], axis=mybir.AxisListType.X)
            rc = small.tile([128, BL, 1], f32, tag="rc")
            nc.vector.reciprocal(rc[:], sm[:])
            nc.vector.tensor_mul(cpad[:, :, sc, 0:16], ex[:],
                                 rc.broadcast_to([128, BL, 16]))

    # ---------- out[b, j, d] = v32[q = j*8 + b, d + 64*(j%2)] ----------
    for j in range(NCAP):
        out_ap = bass.AP(tensor=out_d.tensor, offset=64 * j,
                         ap=[[1024, 8], [1, 64]])
        c0 = 64 * (j % 2)
        nc.sync.dma_start(out=out_ap,
                          in_=v32[8 * j:8 * j + 8, c0:c0 + 64])
    ctx.close()


_CACHE = {}


def _host_consts():
    ident = np.ascontiguousarray(np.eye(128, dtype=np.float16))
    ident32 = np.ascontiguousarray(np.eye(128, dtype=np.float32))
    eps = np.full((128, 1), 1e-7, np.float32)
    cpad = np.zeros((128, BL, SC, 32), np.float16)
    cpad[:, :, :, 0:16] = 1.0 / NCAP          # iteration-0 softmax is exact
    logi = np.zeros((128, BL, SC, 16), np.float32)
    vblk = np.zeros((128, OC, 128), np.float16)
    tti = np.zeros((128, 128), np.float32)
    return {"id16": ident, "id32": ident32, "epsb": eps,
            "cpadi": cpad, "logi": logi, "vblki": vblk, "tti": tti}


def _get_nc():
    if "nc" not in _CACHE:
        nc = bacc.Bacc("TRN2", target_bir_lowering=False, debug=False)
        x_d = nc.dram_tensor("xh", [128, BL, SC, 1024], f16,
                             kind="ExternalInput")
        xt_d = nc.dram_tensor("xth", [128, BL, HC, 512], f16,
                              kind="ExternalInput")
        w_d = nc.dram_tensor("wh", [128, HC, 1024], f16,
                             kind="ExternalInput")
        wt_d = nc.dram_tensor("wth", [128, OC, 1024], f16,
                              kind="ExternalInput")
        id16_d = nc.dram_tensor("id16", [128, 128], f16,
                                kind="ExternalInput")
        id32_d = nc.dram_tensor("id32", [128, 128], f32,
                                kind="ExternalInput")
        eps_d = nc.dram_tensor("epsb", [128, 1], f32, kind="ExternalInput")
        cpad_d = nc.dram_tensor("cpadi", [128, BL, SC, 32], f16,
                                kind="ExternalInput")
        logits_d = nc.dram_tensor("logi", [128, BL, SC, 16], f32,
                                  kind="ExternalInput")
        vblk_d = nc.dram_tensor("vblki", [128, OC, 128], f16,
                                kind="ExternalInput")
        tt_d = nc.dram_tensor("tti", [128, 128], f32, kind="ExternalInput")
        out_d = nc.dram_tensor("out", [BL, NCAP, DCAP], f32,
                               kind="ExternalOutput")
        with tile.TileContext(nc) as tc:
            _build_kernel(tc, out_d.ap(), x_d.ap(), xt_d.ap(), w_d.ap(),
                          wt_d.ap(), id16_d.ap(), id32_d.ap(), eps_d.ap(),
                          cpad_d.ap(), logits_d.ap(), vblk_d.ap(), tt_d.ap())
        nc.compile()
        _CACHE["nc"] = nc
    return _CACHE["nc"]


def kernel(inputs: np.ndarray, W: np.ndarray, _trace: bool = False):
    """inputs: (512, 64, 1024) f32; W: (1, 1024, 1024) f32.
    Returns (64, 16, 64) f32."""
    nc = _get_nc()
    consts = _host_consts()
    w0 = W[0].astype(np.float16)
    wh = np.ascontiguousarray(w0.reshape(HC, 128, 1024).transpose(1, 0, 2))
    wth = np.ascontiguousarray(w0.reshape(1024, OC, 128).transpose(2, 1, 0))
    xf = inputs.astype(np.float16)              # (512, 64, 1024)
    in_maps = []
    for c in range(N_CORES):
        xs = xf[:, c * BL:(c + 1) * BL, :]      # (512, BL, 1024)
        xh = np.ascontiguousarray(
            xs.reshape(SC, 128, BL, 1024).transpose(1, 2, 0, 3))
        xth = np.ascontiguousarray(
            xs.reshape(512, BL, HC, 128).transpose(3, 1, 2, 0))
        m = {"xh": xh, "xth": xth, "wh": wh, "wth": wth}
        m.update(consts)
        in_maps.append(m)
    kw = {}
    if _trace:
        kw = dict(trace=True, trace_cores=list(range(N_CORES)),
                  stitch_traces=False)
    res = run_bass_kernel_spmd(nc, in_maps, core_ids=list(range(N_CORES)),
                               **kw)
    out = np.concatenate([res.results[c]["out"] for c in range(N_CORES)],
                         axis=0)
    if _trace:
        return out.astype(np.float32), res
    return out.astype(np.float32)


# revision 17
# speedup vs baseline: 1.6943x; 1.1812x over previous
"""Trainium2 Bass kernel for capsule dynamic routing (nn_Capsule).

Reference (per batch item b):
    u = x_b @ W; logits = 0
    for i in 4:
        c = softmax(logits, axis=capsule)
        t_j = sum_s c[s,j] * u[s, j*64:(j+1)*64]; v = squash(t)
        if i < 3: logits[s,j] += u[s, jblk] . v_j

Never materializes u. By linearity:
    y_q   = sum_s c[s,q] x_s        (GEMM over S, X natural layout)
    T     = y @ W, t = blockdiag(T) (dense q=(b,j) partition layout)
    P     = W^T' Vblk               (Vblk = block-diag of v)
    upd   = X P                     (GEMM over H, X^T layout)

Sharding: batch-parallel, 8 batch items per core, W replicated.
Dense partition index q = b*16 + j (8 batch x 16 capsules = 128).

Perf design (vs. the first working version):
  - All big operands are pre-packed on HOST into the exact f16 SBUF
    layouts (x, x^T, W, W^T) -> contiguous HWDGE DMA loads, no SWDGE
    casts, no element-strided on-chip transpose DMAs.
  - T and P are computed in the dense q layout: 16 matmuls of
    N=512/128 per step instead of 64 padded-256 matmuls.
  - squash runs in natural layout (q on partitions, d on free axis):
    DVE reduce + ACT sqrt, no PE broadcast matmuls.

HW lessons kept from the previous version:
  - ScalarE activation(Copy) for every PSUM f32 -> f16 cast (DVE dies).
  - Each PE-transpose output gets its own PSUM tile.
  - matmul start=True lazily zeroes the PSUM bank for the out AP's
    partitions; partition-disjoint groups interleave with
    skip_group_check=True.
  - f16 constants come from host DRAM, not memset.
"""
import numpy as np
from contextlib import ExitStack

import concourse.bass as bass
import concourse.bacc as bacc
import concourse.tile as tile
from concourse import mybir
from concourse.bass_utils import run_bass_kernel_spmd

f16 = mybir.dt.float16
f32 = mybir.dt.float32
COPY = mybir.ActivationFunctionType.Copy
EXP = mybir.ActivationFunctionType.Exp
SQRT = mybir.ActivationFunctionType.Sqrt

S, B, H = 512, 64, 1024
NCAP, DCAP = 16, 64
ROUTINGS = 4
N_CORES = 8
BL = B // N_CORES          # 8 batch items per core
SC = S // 128              # 4 s-chunks
HC = H // 128              # 8 h-chunks
OC = H // 128              # 8 o-chunks (o = NCAP*DCAP = 1024)


def _act_copy(nc, out, in_):
    nc.scalar.activation(out=out, in_=in_, func=COPY, scale=1.0, alpha=0.0)


def _build_kernel(tc, out_d, x_d, xt_d, w_d, wt_d, id16_d, id32_d, eps_d,
                  cpad_d, logits_d, vblk_d, tt_d):
    nc = tc.nc
    ctx = ExitStack()
    const = ctx.enter_context(tc.tile_pool(name="const", bufs=1))
    work = ctx.enter_context(tc.tile_pool(name="work", bufs=2))
    small = ctx.enter_context(tc.tile_pool(name="small", bufs=2))
    # PSUM budget: 8 banks. big: y_ps/T_ps/PT_ps/u_ps cycle one 4-bank
    # slot (bufs=1); tp: all transposes, 4 x 1 bank.
    ps_big = ctx.enter_context(tc.tile_pool(name="ps_big", bufs=1,
                                            space="PSUM"))
    ps_tp = ctx.enter_context(tc.tile_pool(name="ps_tp", bufs=4, space="PSUM"))

    # ---------- persistent tensors ----------
    x16 = const.tile([128, BL, SC, 1024], f16)    # X natural (s_loc, b, sc, h)
    xt16 = const.tile([128, BL, HC, 512], f16)    # X^T (h_loc, b, hc, s)
    w16 = const.tile([128, HC, 1024], f16)        # W natural (h_loc, hc, o)
    wt16 = const.tile([128, OC, 1024], f16)       # W^T (o_loc, oc, h)
    id16 = const.tile([128, 128], f16)            # eye(128) f16
    id32 = const.tile([128, 128], f32)            # eye(128) f32
    eps = const.tile([128, 1], f32)
    cpad = const.tile([128, BL, SC, 32], f16)     # c, cols 16-31 zero pad
    logits = const.tile([128, BL, SC, 16], f32)
    vblk = const.tile([128, OC, 128], f16)        # block-diag v, dense q cols
    tt16 = const.tile([128, 128], f16)            # t^T split, zeros elsewhere

    # ---------- loads (all contiguous, host-prepacked f16) ----------
    # Small consts FIRST so iteration-0 compute can ride the x16 chunks.
    nc.sync.dma_start(out=id16[:], in_=id16_d[:])
    nc.sync.dma_start(out=id32[:], in_=id32_d[:])
    nc.sync.dma_start(out=eps[:], in_=eps_d[:])
    nc.sync.dma_start(out=cpad[:], in_=cpad_d[:])
    nc.sync.dma_start(out=logits[:], in_=logits_d[:])
    nc.sync.dma_start(out=vblk[:], in_=vblk_d[:])
    nc.sync.dma_start(out=tt16[:], in_=tt_d[:])
    for sc in range(SC):
        for b in range(BL):
            nc.sync.dma_start(out=x16[:, b, sc, :], in_=x_d[:, b, sc, :])
    nc.sync.dma_start(out=w16[:], in_=w_d[:])
    nc.sync.dma_start(out=wt16[:], in_=wt_d[:])
    for b in range(BL):
        nc.sync.dma_start(out=xt16[:, b, :, :], in_=xt_d[:, b, :, :])

    v32 = None
    for it in range(ROUTINGS):
        last = it == ROUTINGS - 1

        # ---------- y = C^T X: (2g x 4b x 32pad part, 1024 h) f32 ----------
        y_ps = ps_big.tile([128, 2, 1024], f32, tag="big", name=f"y_ps{it}")
        for g in range(2):
            for half in range(2):
                for sc in range(SC):
                    for b_ in range(4):
                        b = 4 * g + b_
                        nc.tensor.matmul(
                            y_ps[32 * b_:32 * b_ + 32, g,
                                 512 * half:512 * half + 512],
                            cpad[:, b, sc, :],
                            x16[:, b, sc, 512 * half:512 * half + 512],
                            start=(sc == 0), stop=(sc == SC - 1),
                            skip_group_check=True,
                            tile_position=(0, 32 * b_))
        y_sb = work.tile([128, 2, 1024], f16, tag="y_sb")
        for g in range(2):
            for half in range(2):
                _act_copy(nc, y_sb[:, g, 512 * half:512 * half + 512],
                          y_ps[:, g, 512 * half:512 * half + 512])

        # ---------- y^T via PE transposes, dense-packed q cols ----------
        yt = work.tile([128, HC, 128], f16, tag="yt")
        for hc in range(HC):
            for g in range(2):
                tp = ps_tp.tile([128, 128], f16, tag="tp",
                                name=f"yt_tp{it}_{hc}_{g}")
                nc.tensor.matmul(
                    tp[:], y_sb[:, g, 128 * hc:128 * hc + 128], id16[:],
                    is_transpose=True, skip_group_check=True)
                # dense capsule-major pack: yt col q = j*8 + b
                src = tp.rearrange("p (b j) -> p j b", j=32)[:, 0:16, :]
                dst = yt[:, hc, :].rearrange(
                    "p (j b) -> p j b", b=8)[:, :, 4 * g:4 * g + 4]
                if (hc + 2 * g) % 2 == 0:
                    nc.vector.tensor_copy(dst, src)
                else:
                    _act_copy(nc, dst, src)

        # ---------- T = y W: (128 q, 1024 o) f32, dense ----------
        t_ps = ps_big.tile([128, 1024], f32, tag="big", name=f"t_ps{it}")
        for half in range(2):
            for hc in range(HC):
                nc.tensor.matmul(
                    t_ps[:, 512 * half:512 * half + 512],
                    yt[:, hc, :],
                    w16[:, hc, 512 * half:512 * half + 512],
                    start=(hc == 0), stop=(hc == HC - 1))
        t_sb16 = work.tile([128, 1024], f16, tag="t_sb16")
        for half in range(2):
            _act_copy(nc, t_sb16[:, 512 * half:512 * half + 512],
                      t_ps[:, 512 * half:512 * half + 512])

        # ---------- T^T chunks via PE transpose; aligned extracts -------
        # tt16[par*64 + d, q] = t[q, d] for q in [8j, 8j+8), j = 2oc+par;
        # other cells stay zero (host-initialized).
        for oc in range(OC):
            ttp = ps_tp.tile([128, 128], f16, tag="tp",
                             name=f"tt_tp{it}_{oc}")
            nc.tensor.matmul(ttp[:], t_sb16[:, 128 * oc:128 * oc + 128],
                             id16[:], is_transpose=True,
                             skip_group_check=True)
            for par in range(2):
                j = 2 * oc + par
                p0 = 64 * par
                nc.vector.tensor_copy(
                    tt16[p0:p0 + 64, 8 * j:8 * j + 8],
                    ttp[p0:p0 + 64, 8 * j:8 * j + 8])

        # ---------- transpose to t[q, d'] (d' = d + 64*(j%2)) ----------
        ttq = ps_tp.tile([128, 128], f16, tag="tp", name=f"ttq{it}")
        nc.tensor.matmul(ttq[:], tt16[:], id16[:],
                         is_transpose=True, skip_group_check=True)
        t_sb = small.tile([128, 128], f16, tag="t_sb")
        nc.vector.tensor_copy(t_sb[:], ttq[:])

        # ---------- squash: v = t / sqrt(sum_d t^2 + eps) ----------
        t2 = small.tile([128, 128], f32, tag="t2")
        nc.vector.tensor_mul(t2[:], t_sb[:], t_sb[:])
        ssum = small.tile([128, 1], f32, tag="ssum")
        nc.vector.reduce_sum(ssum[:], t2[:], axis=mybir.AxisListType.X)
        snorm = small.tile([128, 1], f32, tag="snorm")
        nc.scalar.activation(out=snorm[:], in_=ssum[:], func=SQRT,
                             bias=eps[:], scale=1.0, alpha=0.0)
        rs = small.tile([128, 1], f32, tag="rs")
        nc.vector.reciprocal(rs[:], snorm[:])
        if last:
            v32 = small.tile([128, 128], f32, tag="v32")
            nc.vector.tensor_mul(v32[:], t_sb[:],
                                 rs.broadcast_to([128, 128]))
            break
        v16 = small.tile([128, 128], f16, tag="v16")
        nc.vector.tensor_mul(v16[:], t_sb[:], rs.broadcast_to([128, 128]))

        # ---------- scatter v into block-diag vblk (via transpose) -------
        vtp = ps_tp.tile([128, 128], f16, tag="tp", name=f"v_tp{it}")
        nc.tensor.matmul(vtp[:], v16[:], id16[:],
                         is_transpose=True, skip_group_check=True)
        for oc in range(OC):
            for par in range(2):
                j = 2 * oc + par
                p0, p1 = 64 * par, 64 * par + 64
                nc.vector.tensor_copy(vblk[p0:p1, oc, 8 * j:8 * j + 8],
                                      vtp[p0:p1, 8 * j:8 * j + 8])

        # ---------- P^T = Vblk^T W^T: (128 q, 1024 h) f32, dense -------
        pt_ps = ps_big.tile([128, 1024], f32, tag="big", name=f"pt_ps{it}")
        for half in range(2):
            for oc in range(OC):
                nc.tensor.matmul(
                    pt_ps[:, 512 * half:512 * half + 512],
                    vblk[:, oc, :],
                    wt16[:, oc, 512 * half:512 * half + 512],
                    start=(oc == 0), stop=(oc == OC - 1))
        pt_sb = work.tile([128, 1024], f16, tag="pt_sb")
        for half in range(2):
            _act_copy(nc, pt_sb[:, 512 * half:512 * half + 512],
                      pt_ps[:, 512 * half:512 * half + 512])

        # ---------- transpose P^T -> P natural, batch-major cols --------
        p_sb = work.tile([128, HC, 128], f16, tag="p_sb")
        for hc in range(HC):
            ptp = ps_tp.tile([128, 128], f16, tag="tp",
                             name=f"p_tp{it}_{hc}")
            nc.tensor.matmul(
                ptp[:], pt_sb[:, 128 * hc:128 * hc + 128], id16[:],
                is_transpose=True, skip_group_check=True)
            # permute cols capsule-major q=j*8+b -> batch-major b*16+j
            nc.vector.tensor_copy(
                p_sb[:, hc, :].rearrange("p (b j) -> p j b", j=16),
                ptp.rearrange("p (j b) -> p j b", b=8))

        # ---------- update = X P via X^T (dense 16-col weights) ----------
        u_ps = ps_big.tile([128, 2, 512], f32, tag="big", name=f"u_ps{it}")
        for g in range(2):
            for hc in range(HC):
                for b_ in range(4):
                    b = 4 * g + b_
                    nc.tensor.matmul(
                        u_ps[32 * b_:32 * b_ + 16, g, :],
                        p_sb[:, hc, 16 * b:16 * b + 16],
                        xt16[:, b, hc, :],
                        start=(hc == 0), stop=(hc == HC - 1),
                        skip_group_check=True,
                        tile_position=(0, 32 * b_))
        u_sb = work.tile([128, 2, 512], f32, tag="u_sb")
        for g in range(2):
            nc.vector.tensor_copy(u_sb[:, g, :], u_ps[:, g, :])

        # ---------- transpose update, accumulate logits ----------
        for sc in range(SC):
            for g in range(2):
                utp = ps_tp.tile([128, 128], f32, tag="tp",
                                 name=f"ut_tp{it}_{sc}_{g}")
                nc.tensor.matmul(
                    utp[:], u_sb[:, g, 128 * sc:128 * sc + 128], id32[:],
                    is_transpose=True, skip_group_check=True)
                src = utp.rearrange("p (b j) -> p b j", j=32)
                nc.vector.tensor_add(
                    logits[:, 4 * g:4 * g + 4, sc, :],
                    logits[:, 4 * g:4 * g + 4, sc, :], src[:, :, 0:16])

        # ---------- softmax over capsules -> cpad ----------
        for sc in range(SC):
            ex = small.tile([128, BL, 16], f32, tag="ex")
            nc.scalar.activation(out=ex[:], in_=logits[:, :, sc, :],
                                 func=EXP, scale=1.0, alpha=0.0)
            sm = small.tile([128, BL, 1], f32, tag="sm")
            nc.vector.reduce_sum(sm[:], ex[:], axis=mybir.AxisListType.X)
            rc = small.tile([128, BL, 1], f32, tag="rc")
            nc.vector.reciprocal(rc[:], sm[:])
            nc.vector.tensor_mul(cpad[:, :, sc, 0:16], ex[:],
                                 rc.broadcast_to([128, BL, 16]))

    # ---------- out[b, j, d] = v32[q = j*8 + b, d + 64*(j%2)] ----------
    for j in range(NCAP):
        out_ap = bass.AP(tensor=out_d.tensor, offset=64 * j,
                         ap=[[1024, 8], [1, 64]])
        c0 = 64 * (j % 2)
        nc.sync.dma_start(out=out_ap,
                          in_=v32[8 * j:8 * j + 8, c0:c0 + 64])
    ctx.close()


_CACHE = {}


def _host_consts():
    ident = np.ascontiguousarray(np.eye(128, dtype=np.float16))
    ident32 = np.ascontiguousarray(np.eye(128, dtype=np.float32))
    eps = np.full((128, 1), 1e-7, np.float32)
    cpad = np.zeros((128, BL, SC, 32), np.float16)
    cpad[:, :, :, 0:16] = 1.0 / NCAP          # iteration-0 softmax is exact
    logi = np.zeros((128, BL, SC, 16), np.float32)
    vblk = np.zeros((128, OC, 128), np.float16)
    tti = np.zeros((128, 128), np.float16)
    return {"id16": ident, "id32": ident32, "epsb": eps,
            "cpadi": cpad, "logi": logi, "vblki": vblk, "tti": tti}


def _get_nc():
    if "nc" not in _CACHE:
        nc = bacc.Bacc("TRN2", target_bir_lowering=False, debug=False)
        x_d = nc.dram_tensor("xh", [128, BL, SC, 1024], f16,
                             kind="ExternalInput")
        xt_d = nc.dram_tensor("xth", [128, BL, HC, 512], f16,
                              kind="ExternalInput")
        w_d = nc.dram_tensor("wh", [128, HC, 1024], f16,
                             kind="ExternalInput")
        wt_d = nc.dram_tensor("wth", [128, OC, 1024], f16,
                              kind="ExternalInput")
        id16_d = nc.dram_tensor("id16", [128, 128], f16,
                                kind="ExternalInput")
        id32_d = nc.dram_tensor("id32", [128, 128], f32,
                                kind="ExternalInput")
        eps_d = nc.dram_tensor("epsb", [128, 1], f32, kind="ExternalInput")
        cpad_d = nc.dram_tensor("cpadi", [128, BL, SC, 32], f16,
                                kind="ExternalInput")
        logits_d = nc.dram_tensor("logi", [128, BL, SC, 16], f32,
                                  kind="ExternalInput")
        vblk_d = nc.dram_tensor("vblki", [128, OC, 128], f16,
                                kind="ExternalInput")
        tt_d = nc.dram_tensor("tti", [128, 128], f16, kind="ExternalInput")
        out_d = nc.dram_tensor("out", [BL, NCAP, DCAP], f32,
                               kind="ExternalOutput")
        with tile.TileContext(nc) as tc:
            _build_kernel(tc, out_d.ap(), x_d.ap(), xt_d.ap(), w_d.ap(),
                          wt_d.ap(), id16_d.ap(), id32_d.ap(), eps_d.ap(),
                          cpad_d.ap(), logits_d.ap(), vblk_d.ap(), tt_d.ap())
        nc.compile()
        _CACHE["nc"] = nc
    return _CACHE["nc"]


def kernel(inputs: np.ndarray, W: np.ndarray, _trace: bool = False):
    """inputs: (512, 64, 1024) f32; W: (1, 1024, 1024) f32.
    Returns (64, 16, 64) f32."""
    nc = _get_nc()
    consts = _host_consts()
    w0 = W[0].astype(np.float16)
    wh = np.ascontiguousarray(w0.reshape(HC, 128, 1024).transpose(1, 0, 2))
    wth = np.ascontiguousarray(w0.reshape(1024, OC, 128).transpose(2, 1, 0))
    xf = inputs.astype(np.float16)              # (512, 64, 1024)
    in_maps = []
    for c in range(N_CORES):
        xs = xf[:, c * BL:(c + 1) * BL, :]      # (512, BL, 1024)
        xh = np.ascontiguousarray(
            xs.reshape(SC, 128, BL, 1024).transpose(1, 2, 0, 3))
        xth = np.ascontiguousarray(
            xs.reshape(512, BL, HC, 128).transpose(3, 1, 2, 0))
        m = {"xh": xh, "xth": xth, "wh": wh, "wth": wth}
        m.update(consts)
        in_maps.append(m)
    kw = {}
    if _trace:
        kw = dict(trace=True, trace_cores=list(range(N_CORES)),
                  stitch_traces=False)
    res = run_bass_kernel_spmd(nc, in_maps, core_ids=list(range(N_CORES)),
                               **kw)
    out = np.concatenate([res.results[c]["out"] for c in range(N_CORES)],
                         axis=0)
    if _trace:
        return out.astype(np.float32), res
    return out.astype(np.float32)
